# revision 1
# baseline (speedup 1.0000x reference)
"""Trainium2 Bass kernel for nn_ContrastiveLoss (circular-shift negatives).

Reference computation (B=4096, D=1024, S=5):
    d_p[k]      = ||v[k] - a[k] + eps||
    d_n1[k,m]   = ||v[k] - a[idx(k,m)] + eps||,  idx(k,m) = (k+m+1)%B  (m==k -> (k+1)%B)
    d_n2[k,m]   = ||a[k] - v[idx(k,m)] + eps||
    loss        = mean(relu(1 + 2*d_p - min_m d_n1 - min_m d_n2))

Strategy (8 cores, data-parallel over batch, 512 anchors/core + 5-row halo):
  - All distances via the norm expansion ||x-y||^2 = ||x||^2 + ||y||^2 - 2<x,y>.
    (The +eps inside the norm perturbs d^2 by ~1e-4 relative 2e-8 -> dropped;
     effect on the result is ~1e-6 relative, far below tolerance.)
  - <v[k], a[j]> for the band j in [k, k+5] plus row norms come from PE
    matmuls over transposed tiles: band1 = [V.A^T | V.V^T], band2 = [A.A^T | A.V^T]
    computed per 128-anchor block with a 2-group rhs access pattern (N=266)
    so float32r runs at 1 cycle/row.
  - Diagonal extraction: bands are bounced SBUF->DRAM, then strided DMA
    gathers (element stride 1066 = row pitch + 2) pull the 11 diagonals into
    lane-aligned [128, m] tiles. Small vector/scalar epilogue computes the
    hinge. Row norms ride along as the VV/AA diagonals of the same bands.
  - Anchors k<5 (where m==k rewrites the negative index) are recomputed
    exactly on the host in numpy and spliced in.
"""

import numpy as np

B, D, S = 4096, 1024, 5
NCORES = 8
SH = B // NCORES          # 512 anchors per core
ROWS = SH + S             # 517 rows needed per shard (incl. halo)
MARGIN = 1.0
EPS = 1e-6

_CACHE = {}


def _build():
    import concourse.bass as bass
    import concourse.bacc as bacc
    import concourse.tile as tile
    import concourse.mybir as mybir
    from concourse.masks import make_identity

    f32 = mybir.dt.float32
    f32r = mybir.dt.float32r

    nc = bacc.Bacc()
    v_ext = nc.declare_dram_parameter("v", [ROWS, D], f32, isOutput=False)
    a_ext = nc.declare_dram_parameter("a", [ROWS, D], f32, isOutput=False)
    loss_ext = nc.declare_dram_parameter("loss", [SH], f32, isOutput=True)

    NB = SH // 128            # 4 anchor blocks per core
    NC = D // 128             # 8 contraction chunks
    W = 520                   # column pitch of one tensor group in T_all
    BW = 133                  # band width per group (128 anchors + 5 halo)
    N2 = 2 * BW               # 266 = matmul moving free size (>=256 -> f32r fast)

    from contextlib import ExitStack

    with tile.TileContext(nc) as tc, ExitStack() as ctx:
        sing = ctx.enter_context(tc.tile_pool(name="sing", bufs=1))
        natp = ctx.enter_context(tc.tile_pool(name="natp", bufs=1))
        tallp = ctx.enter_context(tc.tile_pool(name="tall", bufs=1))
        tpsum = ctx.enter_context(tc.tile_pool(name="tpsum", bufs=5, space="PSUM"))
        hpsum = ctx.enter_context(tc.tile_pool(name="hpsum", bufs=1, space="PSUM"))
        bpsum = ctx.enter_context(tc.tile_pool(name="bpsum", bufs=2, space="PSUM"))
        dramp = ctx.enter_context(tc.tile_pool(name="dramp", bufs=2, space="DRAM"))
        ndram = ctx.enter_context(tc.tile_pool(name="ndram", bufs=1, space="DRAM"))
        ep = ctx.enter_context(tc.tile_pool(name="ep", bufs=1))

        identity = sing.tile([128, 128], f32, tag="ident")
        make_identity(nc, identity)

        # preload the ACT Sqrt table early so the epilogue doesn't stall on
        # a ~1.3us LoadActFuncSet.
        warm = sing.tile([128, 1], f32, tag="warm")
        nc.vector.memset(warm[:, :], 1.0)
        nc.scalar.activation(out=warm[:, :], in_=warm[:, :],
                             func=mybir.ActivationFunctionType.Sqrt)

        # T_all[p, c, 2j+s]: s=0 -> A^T col j, s=1 -> V^T col j (interleaved
        # so each block's 266-wide rhs window is one contiguous slice, as
        # required by the fp32r matmul ISA restrictions).
        t_all = tallp.tile([128, NC, 2 * W], f32r, tag="t_all")
        t_view = t_all[:, :, :].rearrange("p c (j s) -> p c j s", s=2)

        # ---- loads, transposes and bands, pipelined by 128-row group ----
        # Halo first: tiny and unblocks everything else early.
        halos = {}
        for ti, ext in ((0, a_ext), (1, v_ext)):
            halo = natp.tile([128, D], f32, tag=f"halo{ti}")
            (nc.sync if ti == 0 else nc.scalar).dma_start(out=halo[:5, :], in_=ext[512:ROWS, :])
            halos[ti] = halo

        # halo row norms (rows 512..516) directly from natural layout
        nvh = ep.tile([128, 1], f32, tag="nvh")
        nah = ep.tile([128, 1], f32, tag="nah")
        scr_h = ep.tile([128, D], f32, tag="scr_h")
        scr_h2 = ep.tile([128, D], f32, tag="scr_h2")
        A = mybir.AluOpType
        nc.scalar.activation(out=scr_h[:5, :], in_=halos[0][:5, :],
                             func=mybir.ActivationFunctionType.Square,
                             accum_out=nah[:5, :])
        nc.scalar.activation(out=scr_h2[:5, :], in_=halos[1][:5, :],
                             func=mybir.ActivationFunctionType.Square,
                             accum_out=nvh[:5, :])

        # halo transposes: rows 512..516 -> cols 512..516 of every chunk
        for ti in range(2):
            hps = hpsum.tile([128, 5 * NC], f32, tag="hps")
            for c in range(NC):
                nc.tensor.transpose(
                    hps[:, c * 5 : c * 5 + 5],
                    halos[ti][:5, c * 128 : (c + 1) * 128],
                    identity[:5, :5],
                )
            hview = hps[:, :].rearrange("p (c j) -> p c j", j=5)
            dsth = t_view[:, :, 512:517, ti]
            nc.vector.tensor_copy(dsth, hview)

        def band_matmul(lhs_j0, lhs_s, lhs_n, k0, tag):
            """PSUM [lhs_n, 266] = sum_c lhsT_c.T @ interleaved[A^T|V^T] window.

            lhsT = rows lhs_j0..lhs_j0+lhs_n of tensor lhs_s (0=A, 1=V);
            rhs  = contiguous cols 2*k0 .. 2*k0+265 (row-pairs k0..k0+132).
            Output col 2c+s = <lhs_row, (A if s==0 else V)[k0+c]>.
            """
            bp = bpsum.tile([128, N2], f32, tag="band")
            for c in range(NC):
                lhsT = t_view[:, c, lhs_j0 : lhs_j0 + lhs_n, lhs_s]
                rhs = t_all[:, c, 2 * k0 : 2 * k0 + N2]
                nc.tensor.matmul(bp[:lhs_n, :], lhsT, rhs, start=(c == 0), stop=(c == NC - 1))
            return bp

        b1acc = ep.tile([128, NB, N2], f32, tag="b1acc")
        b2acc = ep.tile([128, NB, N2], f32, tag="b2acc")
        bd1 = dramp.tile([128, NB, N2], f32, tag="bd1")
        bd2 = dramp.tile([128, NB, N2], f32, tag="bd2")
        # g1: j=0 dp dot, j=1 VV diag (nv), j=2,4,6,8,10 dn1 dots
        g1 = ep.tile([128, NB, 11], f32, tag="g1")
        # g2: j=0 AA diag (na), j=3,5,7,9,11 dn2 dots
        g2 = ep.tile([128, NB, 12], f32, tag="g2")
        nv_d = ndram.tile([ROWS + 3], f32, tag="nv_d")
        na_d = ndram.tile([ROWS + 3], f32, tag="na_d")

        def emit_band(b):
            k0 = 128 * b
            bp1 = band_matmul(k0, 1, 128, k0, f"b1_{b}")   # lhsT=V -> VA | VV
            bp2 = band_matmul(k0, 0, 128, k0, f"b2_{b}")   # lhsT=A -> AA | AV
            nc.vector.tensor_copy(b1acc[:, b, :], bp1[:, :])
            nc.scalar.copy(b2acc[:, b, :], bp2[:, :])

        def vec_ap(dram_tile, i0, dims):
            return bass.AP(tensor=dram_tile.tensor, offset=dram_tile.offset + i0, ap=dims)

        # bd layout flat(p, b, c) = 1064*p + 266*b + c; diagonal element
        # (p, b, j) of block b sits at c = 2p + j -> flat = 1066*p + 266*b + j.
        def block_gather(bdt, b, width):
            return bass.AP(
                tensor=bdt.tensor,
                offset=bdt.offset + 266 * b,
                ap=[[1066, 128], [1, width]],
            )

        def emit_extract(b, writeback=True):
            nc.sync.dma_start(out=bd1[:, b, :], in_=b1acc[:, b, :])
            nc.scalar.dma_start(out=bd2[:, b, :], in_=b2acc[:, b, :])
            nc.sync.dma_start(out=g1[:, b, :], in_=block_gather(bd1, b, 11))
            nc.scalar.dma_start(out=g2[:, b, :], in_=block_gather(bd2, b, 12))
            if writeback:
                # norms to the flat staging vectors, straight DRAM->DRAM
                # (VV diag at col 2p+1, AA diag at col 2p)
                nc.sync.dma_start(
                    out=vec_ap(nv_d, 128 * b, [[1, 128]]),
                    in_=bass.AP(tensor=bd1.tensor, offset=bd1.offset + 266 * b + 1,
                                ap=[[1066, 128]]),
                )
                nc.scalar.dma_start(
                    out=vec_ap(na_d, 128 * b, [[1, 128]]),
                    in_=bass.AP(tensor=bd2.tensor, offset=bd2.offset + 266 * b,
                                ap=[[1066, 128]]),
                )

        # halo norms land early (computed from natural tiles above)
        nc.sync.dma_start(out=vec_ap(nv_d, 512, [[1, 5]]), in_=nvh[:5, :])
        nc.scalar.dma_start(out=vec_ap(na_d, 512, [[1, 5]]), in_=nah[:5, :])

        # block-3 norms come from a direct ACT square+accum on the rg3 natural
        # tiles (cheap, lands mid-timeline) so the shifted gathers don't have
        # to wait for block 3's band extract.
        norma3 = ep.tile([128, 1], f32, tag="norma3")
        normv3 = ep.tile([128, 1], f32, tag="normv3")
        scr_n0 = ep.tile([128, D], f32, tag="scr_n0")
        scr_n1 = ep.tile([128, D], f32, tag="scr_n1")
        scr_n = [scr_n0, scr_n1]

        # Row-group loop: load rg -> transpose rg -> band (b = rg-1) as soon
        # as its inputs (row groups <= b+1) are in T_all.
        for rg in range(4):
            nats_rg = {}
            for ti, ext in ((0, a_ext), (1, v_ext)):
                nat = natp.tile([128, D], f32, tag=f"nat{ti}_{rg}")
                eng = nc.sync if (rg + ti) % 2 == 0 else nc.scalar
                eng.dma_start(out=nat[:, :], in_=ext[rg * 128 : (rg + 1) * 128, :])
                nats_rg[ti] = nat
                if rg == 3:
                    nacc = norma3 if ti == 0 else normv3
                    nc.scalar.activation(out=scr_n[ti][:, :], in_=nat[:, :],
                                         func=mybir.ActivationFunctionType.Square,
                                         accum_out=nacc[:, :])
                    eng.dma_start(out=vec_ap(nv_d if ti else na_d, 384, [[1, 128]]),
                                  in_=nacc[:, :])
            for ti in range(2):
                for half in range(2):
                    ps = tpsum.tile([128, 512], f32, tag="tps")
                    for ci in range(4):
                        c = half * 4 + ci
                        nc.tensor.transpose(
                            ps[:, ci * 128 : (ci + 1) * 128],
                            nats_rg[ti][:, c * 128 : (c + 1) * 128],
                            identity,
                        )
                    dst = t_view[:, half * 4 : half * 4 + 4, rg * 128 : (rg + 1) * 128, ti]
                    psv = ps[:, :].rearrange("p (c j) -> p c j", j=128)
                    if (ti + half) % 2 == 0:
                        nc.vector.tensor_copy(dst, psv)
                    else:
                        nc.scalar.copy(dst, psv)
            if rg >= 1:
                emit_band(rg - 1)
                emit_extract(rg - 1)
        # staging vector complete after extracts 0..2 + rg3 norms + halo:
        # fetch shifted views + per-row norm columns, overlapped with band 3
        nash = ep.tile([128, NB, 5], f32, tag="nash")  # ||a[k+1+m]||^2
        nvsh = ep.tile([128, NB, 5], f32, tag="nvsh")  # ||v[k+1+m]||^2
        nvcol = ep.tile([128, NB, 1], f32, tag="nvcol")
        nacol = ep.tile([128, NB, 1], f32, tag="nacol")
        nc.sync.dma_start(out=nash[:, :, :], in_=vec_ap(na_d, 1, [[1, 128], [128, NB], [1, 5]]))
        nc.scalar.dma_start(out=nvsh[:, :, :], in_=vec_ap(nv_d, 1, [[1, 128], [128, NB], [1, 5]]))
        nc.sync.dma_start(out=nvcol[:, :, :], in_=vec_ap(nv_d, 0, [[1, 128], [128, NB]]))
        nc.scalar.dma_start(out=nacol[:, :, :], in_=vec_ap(na_d, 0, [[1, 128], [128, NB]]))
        emit_band(3)
        emit_extract(3, writeback=False)

        # ---- epilogue (per block, so it overlaps remaining bands) ----
        dsq = ep.tile([128, NB, 11], f32, tag="dsq")
        dall = ep.tile([128, NB, 11], f32, tag="dall")
        dn1m = ep.tile([128, NB], f32, tag="dn1m")
        dn2m = ep.tile([128, NB], f32, tag="dn2m")
        tsum = ep.tile([128, NB], f32, tag="tsum")
        hpre = ep.tile([128, NB], f32, tag="hpre")
        lossn = ep.tile([128, NB], f32, tag="lossn")

        nv_t = nvcol[:, :, :]
        na_t = nacol[:, :, :]
        dn1_dots = g1[:, :, 1:11].rearrange("p b (j s) -> p b j s", s=2)[:, :, :, 1]
        dn2_dots = g2[:, :, 2:12].rearrange("p b (j s) -> p b j s", s=2)[:, :, :, 1]

        # shift bases precomputed as soon as nash/nvsh land (overlaps bands)
        base1 = ep.tile([128, NB, 5], f32, tag="base1")
        base2 = ep.tile([128, NB, 5], f32, tag="base2")
        nc.vector.tensor_add(base1[:, :, :], nash[:, :, :],
                             nv_t.broadcast_to([128, NB, 5]))
        nc.vector.tensor_add(base2[:, :, :], nvsh[:, :, :],
                             na_t.broadcast_to([128, NB, 5]))

        def epilogue(b):
            bs = slice(b, b + 1)
            # dn1^2 = -2*dot + (nv + na_shift);  dn2^2 = -2*dot + (na + nv_shift)
            nc.vector.tensor_scalar(out=dsq[:, bs, 0:5], in0=dn1_dots[:, bs, :],
                                    scalar1=-2.0, scalar2=None, op0=A.mult)
            nc.vector.tensor_add(dsq[:, bs, 0:5], dsq[:, bs, 0:5], base1[:, bs, :])
            nc.vector.tensor_scalar(out=dsq[:, bs, 5:10], in0=dn2_dots[:, bs, :],
                                    scalar1=-2.0, scalar2=None, op0=A.mult)
            nc.vector.tensor_add(dsq[:, bs, 5:10], dsq[:, bs, 5:10], base2[:, bs, :])
            # dp^2 = (-2*dot + nv) + na   (scalar2 is per-partition within a block)
            nc.vector.tensor_scalar(out=dsq[:, bs, 10:11], in0=g1[:, bs, 0:1],
                                    scalar1=-2.0, scalar2=nv_t[:, b, :],
                                    op0=A.mult, op1=A.add)
            nc.vector.tensor_add(dsq[:, bs, 10:11], dsq[:, bs, 10:11], na_t[:, bs, :])
            nc.scalar.activation(out=dall[:, bs, :], in_=dsq[:, bs, :],
                                 func=mybir.ActivationFunctionType.Sqrt)
            nc.vector.tensor_reduce(out=dn1m[:, bs], in_=dall[:, bs, 0:5],
                                    axis=mybir.AxisListType.X, op=A.min)
            nc.vector.tensor_reduce(out=dn2m[:, bs], in_=dall[:, bs, 5:10],
                                    axis=mybir.AxisListType.X, op=A.min)
            nc.vector.tensor_add(tsum[:, bs], dn1m[:, bs], dn2m[:, bs])
            nc.vector.tensor_scalar(out=hpre[:, bs], in0=dall[:, bs, 10], scalar1=2.0,
                                    scalar2=MARGIN, op0=A.mult, op1=A.add)
            nc.vector.tensor_sub(lossn[:, bs], hpre[:, bs], tsum[:, bs])
            nc.vector.tensor_scalar_max(out=lossn[:, bs], in0=lossn[:, bs], scalar1=0.0)
            eng = nc.sync if b % 2 == 0 else nc.scalar
            eng.dma_start(
                out=bass.AP(tensor=loss_ext, offset=128 * b, ap=[[1, 128]]),
                in_=lossn[:, bs],
            )

        for b in range(NB):
            epilogue(b)

    nc.finalize()
    return nc


def _exact_losses_head(vfeat, afeat, ks):
    """Exact reference loss for anchors in ks (handles the m==k index rewrite)."""
    v = vfeat.astype(np.float64)
    a = afeat.astype(np.float64)
    out = []
    for k in ks:
        idx = [(m + k + 1) % B if m != k else (k + 1) % B for m in range(S)]
        d_p = np.sqrt(np.sum((v[k] - a[k] + EPS) ** 2))
        d1 = min(np.sqrt(np.sum((v[k] - a[j] + EPS) ** 2)) for j in idx)
        d2 = min(np.sqrt(np.sum((a[k] - v[j] + EPS) ** 2)) for j in idx)
        out.append(max(MARGIN + 2.0 * d_p - d1 - d2, 0.0))
    return out


def run_kernel(vfeat, afeat, trace=False):
    from concourse.bass_utils import run_bass_kernel_spmd

    vfeat = np.ascontiguousarray(np.asarray(vfeat, dtype=np.float32))
    afeat = np.ascontiguousarray(np.asarray(afeat, dtype=np.float32))

    if "nc" not in _CACHE:
        _CACHE["nc"] = _build()
    nc = _CACHE["nc"]

    in_maps = []
    for c in range(NCORES):
        lo = c * SH
        idx = np.arange(lo, lo + ROWS) % B
        in_maps.append({"v": vfeat[idx], "a": afeat[idx]})

    res = run_bass_kernel_spmd(nc, in_maps, core_ids=list(range(NCORES)), trace=trace)
    losses = np.concatenate([res.results[c]["loss"] for c in range(NCORES)])

    total = float(np.sum(losses[S:], dtype=np.float64))
    total += sum(_exact_losses_head(vfeat, afeat, range(S)))
    mean = np.float32(total / B)
    return np.asarray(mean, dtype=np.float32), res


def kernel(vfeat, afeat):
    out, _ = run_kernel(vfeat, afeat, trace=False)
    return out



# revision 8
# speedup vs baseline: 1.3995x; 1.3995x over previous
"""Trainium2 Bass kernel for nn_ContrastiveLoss (circular-shift negatives).

Reference computation (B=4096, D=1024, S=5):
    d_p[k]      = ||v[k] - a[k] + eps||
    d_n1[k,m]   = ||v[k] - a[idx(k,m)] + eps||,  idx(k,m) = (k+m+1)%B  (m==k -> (k+1)%B)
    d_n2[k,m]   = ||a[k] - v[idx(k,m)] + eps||
    loss        = mean(relu(1 + 2*d_p - min_m d_n1 - min_m d_n2))

Strategy (8 cores, data-parallel over batch, 512 anchors/core + 5-row halo):
  - Distances via ||x-y||^2 = ||x||^2 + ||y||^2 - 2<x,y> (the +eps term is
    ~1e-6 relative -> dropped).
  - Dots <v[k], a[k+s]>, s=0..5 (and the v/a-swapped set) come from PE band
    matmuls in bf16 over transposed tiles: per 128-anchor block,
    band1 = V.A_window^T and band2 = A.V_window^T, each [128, 133], both
    accumulated into one PSUM bank.
  - Bands are copied to SBUF as bf16 and bounced to DRAM with a row pitch of
    270 elements (block stride 270*128), which makes the flat address of the
    (p, p+j) diagonals affine in (block, p, j): two strided DMA gathers fetch
    ALL dot products for all 4 blocks.
  - Row norms come from ACT Square+accum on the natural-layout tiles
    (off the critical path), staged to a flat DRAM vector, and gathered back
    shifted (rows k+1..k+5) with one stride-1 DMA per tensor.
  - Big input loads go through the Pool engine's SWDGE path so the shared
    HWDGE unit only serves the small bounce/gather/stage DMAs (on SP/ACT).
  - Anchors k<5 (m==k index rewrite) are recomputed exactly on the host.
"""

import numpy as np

B, D, S = 4096, 1024, 5
NCORES = 8
SH = B // NCORES          # 512 anchors per core
ROWS = SH + S             # 517 rows needed per shard (incl. halo)
MARGIN = 1.0
EPS = 1e-6

_CACHE = {}


def _build():
    import concourse.bass as bass
    import concourse.bacc as bacc
    import concourse.tile as tile
    import concourse.mybir as mybir
    from concourse.masks import make_identity
    from contextlib import ExitStack

    f32 = mybir.dt.float32
    bf16 = mybir.dt.bfloat16
    A = mybir.AluOpType
    ACTF = mybir.ActivationFunctionType

    nc = bacc.Bacc()
    v_ext = nc.declare_dram_parameter("v", [ROWS, D], f32, isOutput=False)
    a_ext = nc.declare_dram_parameter("a", [ROWS, D], f32, isOutput=False)
    loss_ext = nc.declare_dram_parameter("loss", [SH], f32, isOutput=True)

    NB = SH // 128            # 4 anchor blocks per core
    NC = D // 128             # 8 contraction chunks
    BW = 133                  # band width (128 anchors + 5 halo)
    PITCH = 270               # DRAM bounce row pitch (266 used + 4 pad)
    BLK = PITCH * 128         # bounce block stride = 34560

    with tile.TileContext(nc) as tc, ExitStack() as ctx:
        sing = ctx.enter_context(tc.tile_pool(name="sing", bufs=1))
        natp = ctx.enter_context(tc.tile_pool(name="natp", bufs=1))
        tallp = ctx.enter_context(tc.tile_pool(name="tall", bufs=1))
        tpsum = ctx.enter_context(tc.tile_pool(name="tpsum", bufs=4, space="PSUM"))
        bpsum = ctx.enter_context(tc.tile_pool(name="bpsum", bufs=4, space="PSUM"))
        dramp = ctx.enter_context(tc.tile_pool(name="dramp", bufs=1, space="DRAM"))
        ep = ctx.enter_context(tc.tile_pool(name="ep", bufs=1))
        scrp = ctx.enter_context(tc.tile_pool(name="scrp", bufs=2))
        bsbp = ctx.enter_context(tc.tile_pool(name="bsb", bufs=4))

        def vec_ap(dram_tile, i0, dims):
            return bass.AP(tensor=dram_tile.tensor, offset=dram_tile.offset + i0, ap=dims)

        # ---------------- input DMAs ----------------
        # Big loads on the Pool SWDGE path (keeps HWDGE free); halos on SP.
        nat = {}
        nat[1] = natp.tile([128, NB, D], f32, name="nat_v", tag="nat_v")
        nat[0] = natp.tile([128, NB, D], f32, name="nat_a", tag="nat_a")
        halo = natp.tile([128, D], f32, tag="halo")          # parts 0:5 v, 5:10 a

        nc.sync.dma_start(
            out=halo[0:5, :],
            in_=bass.AP(tensor=v_ext, offset=512 * D, ap=[[D, 5], [1, D]]),
        )
        nc.sync.dma_start(
            out=halo[32:37, :],
            in_=bass.AP(tensor=a_ext, offset=512 * D, ap=[[D, 5], [1, D]]),
        )

        def load_pair(ti, rg0):
            ext = v_ext if ti == 1 else a_ext
            nc.gpsimd.dma_start(
                out=nat[ti][:, rg0 : rg0 + 2, :],
                in_=bass.AP(
                    tensor=ext,
                    offset=rg0 * 128 * D,
                    ap=[[D, 128], [128 * D, 2], [1, D]],
                ),
            )

        load_pair(1, 2)   # v rows 256..511
        load_pair(0, 2)   # a rows 256..511

        identity = sing.tile([128, 128], f32, tag="ident")
        make_identity(nc, identity)

        load_pair(1, 0)   # v rows 0..255
        load_pair(0, 0)   # a rows 0..255

        # preload the ACT Sqrt table so the epilogue doesn't stall on it
        warm = sing.tile([128, 1], f32, tag="warm")
        nc.vector.memset(warm[:, :], 1.0)
        nc.scalar.activation(out=warm[:, :], in_=warm[:, :], func=ACTF.Sqrt)

        # ---------------- norms (ACT square+accum, staged to DRAM) --------
        nv_sb = ep.tile([128, NB], f32, tag="nv_sb")
        na_sb = ep.tile([128, NB], f32, tag="na_sb")
        nh = ep.tile([128, 1], f32, tag="nh")      # parts 0:5 ||v||, 32:37 ||a||
        nv_d = dramp.tile([ROWS + 3], f32, tag="nv_d")
        na_d = dramp.tile([ROWS + 3], f32, tag="na_d")

        def square(in_view, accum_view):
            scr = scrp.tile([128, D], f32, tag="scr")
            np_ = in_view.partition_size()
            nc.scalar.activation(out=scr[0:np_, :], in_=in_view, func=ACTF.Square,
                                 accum_out=accum_view)

        square(halo[0:5, :], nh[0:5, :])
        square(halo[32:37, :], nh[32:37, :])

        # t_all[p, c, t, col]: col j of chunk c of tensor t (0=A, 1=V), bf16
        t_all = tallp.tile([128, NC, 2, 520], bf16, tag="t_all")

        # ---------------- transposes ----------------
        hps = tpsum.tile([128, 512], f32, tag="tps", name="hps")  # halo transposes
        for ti in range(2):
            pbase = 0 if ti == 1 else 32   # halo partitions: v at 0:5, a at 32:37
            for c in range(NC):
                nc.tensor.transpose(
                    hps[:, ti * 40 + c * 5 : ti * 40 + c * 5 + 5],
                    halo[pbase : pbase + 5, c * 128 : (c + 1) * 128],
                    identity[pbase : pbase + 5, pbase : pbase + 5],
                )
        for ti in range(2):
            hview = hps[:, ti * 40 : ti * 40 + 40].rearrange("p (c j) -> p c j", j=5)
            nc.vector.tensor_copy(t_all[:, :, ti, 512:517], hview)

        def transpose_rg(ti, rg):
            for half in range(2):
                ps = tpsum.tile([128, 512], f32, tag="tps")
                for ci in range(4):
                    c = half * 4 + ci
                    nc.tensor.transpose(
                        ps[:, ci * 128 : (ci + 1) * 128],
                        nat[ti][:, rg, c * 128 : (c + 1) * 128],
                        identity,
                    )
                dst = t_all[:, half * 4 : half * 4 + 4, ti, rg * 128 : (rg + 1) * 128]
                psv = ps[:, :].rearrange("p (c j) -> p c j", j=128)
                nc.vector.tensor_copy(dst, psv)

        # ---------------- bands + bounce + gather ----------------
        bd = dramp.tile([NB * BLK], bf16, tag="bd")

        def band(b):
            """PSUM [128, 266]: cols 0:133 = V.A_win^T, 133:266 = A.V_win^T."""
            k0 = 128 * b
            bp1 = bpsum.tile([128, BW], f32, tag="band", name="bp1")
            bp2 = bpsum.tile([128, BW], f32, tag="band", name="bp2")
            for c in range(NC):
                lhsT_v = t_all[:, c, 1, k0 : k0 + 128]
                lhsT_a = t_all[:, c, 0, k0 : k0 + 128]
                rhs_a = t_all[:, c, 0, k0 : k0 + BW]
                rhs_v = t_all[:, c, 1, k0 : k0 + BW]
                nc.tensor.matmul(bp1[:, :], lhsT_v, rhs_a,
                                 start=(c == 0), stop=(c == NC - 1))
                nc.tensor.matmul(bp2[:, :], lhsT_a, rhs_v,
                                 start=(c == 0), stop=(c == NC - 1))
            bsb = bsbp.tile([128, 2 * BW], bf16, tag="bsb")
            nc.vector.tensor_copy(bsb[:, 0:BW], bp1[:, :])
            nc.vector.tensor_copy(bsb[:, BW : 2 * BW], bp2[:, :])
            nc.sync.dma_start(
                out=bass.AP(tensor=bd.tensor, offset=bd.offset + BLK * b,
                            ap=[[PITCH, 128], [1, 2 * BW]]),
                in_=bsb[:, :],
            )

        # order: rgs 2,3 land first -> bands 2,3 early; then 0,1
        for ti in (1, 0):
            transpose_rg(ti, 3)
            transpose_rg(ti, 2)
            square(nat[ti][:, 3, :], (nv_sb if ti else na_sb)[:, 3:4])
            square(nat[ti][:, 2, :], (nv_sb if ti else na_sb)[:, 2:3])
        band(3)
        band(2)
        for ti in (1, 0):
            transpose_rg(ti, 1)
            transpose_rg(ti, 0)
            square(nat[ti][:, 1, :], (nv_sb if ti else na_sb)[:, 1:2])
            square(nat[ti][:, 0, :], (nv_sb if ti else na_sb)[:, 0:1])
        band(1)
        band(0)

        # norm staging writes (ACT HWDGE) + shifted-norm gathers
        nc.scalar.dma_start(out=vec_ap(nv_d, 0, [[1, 128], [128, NB]]),
                            in_=nv_sb[:, :])
        nc.scalar.dma_start(out=vec_ap(nv_d, 512, [[1, 5]]), in_=nh[0:5, :])
        nc.scalar.dma_start(out=vec_ap(na_d, 0, [[1, 128], [128, NB]]),
                            in_=na_sb[:, :])
        nc.scalar.dma_start(out=vec_ap(na_d, 512, [[1, 5]]), in_=nh[32:37, :])

        nash = ep.tile([128, NB, 5], f32, tag="nash")   # ||a[k+j]||^2, j=1..5
        nvsh = ep.tile([128, NB, 5], f32, tag="nvsh")
        nc.scalar.dma_start(out=nash[:, :, :],
                            in_=vec_ap(na_d, 1, [[1, 128], [128, NB], [1, 5]]))
        nc.scalar.dma_start(out=nvsh[:, :, :],
                            in_=vec_ap(nv_d, 1, [[1, 128], [128, NB], [1, 5]]))

        # dot gathers: band1 dots at 271p + j (+BLK*b), band2 at +133
        g1 = ep.tile([128, NB, 6], bf16, tag="g1")
        g2 = ep.tile([128, NB, 6], bf16, tag="g2")
        nc.sync.dma_start(
            out=g1[:, :, :],
            in_=bass.AP(tensor=bd.tensor, offset=bd.offset,
                        ap=[[PITCH + 1, 128], [BLK, NB], [1, 6]]),
        )
        nc.sync.dma_start(
            out=g2[:, :, :],
            in_=bass.AP(tensor=bd.tensor, offset=bd.offset + BW,
                        ap=[[PITCH + 1, 128], [BLK, NB], [1, 6]]),
        )

        # ---------------- epilogue ----------------
        g1f = ep.tile([128, NB, 6], f32, tag="g1f")
        g2f = ep.tile([128, NB, 6], f32, tag="g2f")
        nc.vector.tensor_copy(g1f[:, :, :], g1[:, :, :])
        nc.vector.tensor_copy(g2f[:, :, :], g2[:, :, :])

        nv_col = nv_sb[:, :].rearrange("p (b one) -> p b one", one=1)
        na_col = na_sb[:, :].rearrange("p (b one) -> p b one", one=1)

        base1 = ep.tile([128, NB, 5], f32, tag="base1")
        base2 = ep.tile([128, NB, 5], f32, tag="base2")
        basep = ep.tile([128, NB, 1], f32, tag="basep")
        nc.vector.tensor_add(base1[:, :, :], nash[:, :, :],
                             nv_col.broadcast_to([128, NB, 5]))
        nc.vector.tensor_add(base2[:, :, :], nvsh[:, :, :],
                             na_col.broadcast_to([128, NB, 5]))
        nc.vector.tensor_add(basep[:, :, :], nv_col, na_col)

        dsq = ep.tile([128, NB, 11], f32, tag="dsq")
        dall = ep.tile([128, NB, 11], f32, tag="dall")
        nc.vector.tensor_scalar(out=dsq[:, :, 0:5], in0=g1f[:, :, 1:6],
                                scalar1=-2.0, scalar2=None, op0=A.mult)
        nc.vector.tensor_add(dsq[:, :, 0:5], dsq[:, :, 0:5], base1[:, :, :])
        nc.vector.tensor_scalar(out=dsq[:, :, 5:10], in0=g2f[:, :, 1:6],
                                scalar1=-2.0, scalar2=None, op0=A.mult)
        nc.vector.tensor_add(dsq[:, :, 5:10], dsq[:, :, 5:10], base2[:, :, :])
        nc.vector.tensor_scalar(out=dsq[:, :, 10:11], in0=g1f[:, :, 0:1],
                                scalar1=-2.0, scalar2=None, op0=A.mult)
        nc.vector.tensor_add(dsq[:, :, 10:11], dsq[:, :, 10:11], basep[:, :, :])

        nc.scalar.activation(out=dall[:, :, :], in_=dsq[:, :, :], func=ACTF.Sqrt)

        dn1m = ep.tile([128, NB], f32, tag="dn1m")
        dn2m = ep.tile([128, NB], f32, tag="dn2m")
        tsum = ep.tile([128, NB], f32, tag="tsum")
        hpre = ep.tile([128, NB], f32, tag="hpre")
        lossn = ep.tile([128, NB], f32, tag="lossn")
        nc.vector.tensor_reduce(out=dn1m[:, :], in_=dall[:, :, 0:5],
                                axis=mybir.AxisListType.X, op=A.min)
        nc.vector.tensor_reduce(out=dn2m[:, :], in_=dall[:, :, 5:10],
                                axis=mybir.AxisListType.X, op=A.min)
        nc.vector.tensor_add(tsum[:, :], dn1m[:, :], dn2m[:, :])
        nc.vector.tensor_scalar(out=hpre[:, :], in0=dall[:, :, 10], scalar1=2.0,
                                scalar2=MARGIN, op0=A.mult, op1=A.add)
        nc.vector.tensor_sub(lossn[:, :], hpre[:, :], tsum[:, :])
        nc.vector.tensor_scalar_max(out=lossn[:, :], in0=lossn[:, :], scalar1=0.0)
        nc.sync.dma_start(
            out=bass.AP(tensor=loss_ext, offset=0, ap=[[1, 128], [128, NB]]),
            in_=lossn[:, :],
        )

    nc.finalize()
    return nc


def _exact_losses_head(vfeat, afeat, ks):
    """Exact reference loss for anchors in ks (handles the m==k index rewrite)."""
    v = vfeat.astype(np.float64)
    a = afeat.astype(np.float64)
    out = []
    for k in ks:
        idx = [(m + k + 1) % B if m != k else (k + 1) % B for m in range(S)]
        d_p = np.sqrt(np.sum((v[k] - a[k] + EPS) ** 2))
        d1 = min(np.sqrt(np.sum((v[k] - a[j] + EPS) ** 2)) for j in idx)
        d2 = min(np.sqrt(np.sum((a[k] - v[j] + EPS) ** 2)) for j in idx)
        out.append(max(MARGIN + 2.0 * d_p - d1 - d2, 0.0))
    return out


def run_kernel(vfeat, afeat, trace=False):
    from concourse.bass_utils import run_bass_kernel_spmd

    vfeat = np.ascontiguousarray(np.asarray(vfeat, dtype=np.float32))
    afeat = np.ascontiguousarray(np.asarray(afeat, dtype=np.float32))

    if "nc" not in _CACHE:
        _CACHE["nc"] = _build()
    nc = _CACHE["nc"]

    in_maps = []
    for c in range(NCORES):
        lo = c * SH
        idx = np.arange(lo, lo + ROWS) % B
        in_maps.append({"v": vfeat[idx], "a": afeat[idx]})

    res = run_bass_kernel_spmd(nc, in_maps, core_ids=list(range(NCORES)), trace=trace)
    losses = np.concatenate([res.results[c]["loss"] for c in range(NCORES)])

    total = float(np.sum(losses[S:], dtype=np.float64))
    total += sum(_exact_losses_head(vfeat, afeat, range(S)))
    mean = np.float32(total / B)
    return np.asarray(mean, dtype=np.float32), res


def kernel(vfeat, afeat):
    out, _ = run_kernel(vfeat, afeat, trace=False)
    return out
